# revision 32
# baseline (speedup 1.0000x reference)
"""Multi-head attention kernel for 8 Trainium2 NeuronCores.

Problem: B=16, S=512, D=768, H=12 heads (dk=64), fp32.
  y = softmax(QK^T/sqrt(dk) + mask*(-1e9) + adj) V, with QKV/out projections.

Strategy: data-parallel over batch (2 batches per core). On the host we
pre-transpose activations and weights so the device kernel needs zero
on-device transposes; everything on-device is matmul + softmax arithmetic.
Activations/weights/V/E/xout travel as bf16 (halves DMA + doubles DVE
throughput); Q^T/K^T and all PSUM accumulation stay fp32, so the softmax
argument is computed at fp32r matmul precision.

Device dataflow (per core, per batch, "transposed domain"):
  QT[e,i]  = (Wq/8)T-contracted proj of xqT          (e on partitions, f32r)
  KT[e,i]  = proj of xkT                             (f32r)
  V'[j,e'] = proj of xvT with Wv augmented on the host by one zero column +
             bias 1.0 per head, so each head carries a built-in ones column
             (natural layout: tokens on partitions, e' = h*65 + c)
  per head pair (2h, 2h+1), score matmuls interleaved per jc chunk so the two
  heads run concurrently on disjoint 64-row groups of the PE array:
    S.T[j,i]  = KT_h matmuls (K=dk=64)               -> PSUM
    E0.T[j,i] = exp(S.T)                             (scalar engine, bf16 out)
    E.T[j,i]  = E0.T * exp(adjT + mask*NEG)          (gpsimd/vector split;
                exp(adj+mask*NEG) precomputed on host: exp(a+b)=exp(a)exp(b))
    X'[c,i]  += V'_h attn@V; row 64 = softmax denom l[i]  (M=65)
  l broadcast to 64 partitions by a K=1 matmul, reciprocal_approx_fast on 64
  lanes, normalize during PSUM copyback; odd heads DMA-packed to partitions
  64:128 so the output projection contracts head pairs with K=128.

The two batches are software-pipelined against each other at emission level:
  phase A: batch-0 projections
  phase B: batch-0 attention pairs interleaved with batch-1 projection groups
  phase C: batch-1 attention pairs interleaved with batch-0 out-projection
  phase D: batch-1 out-projection
so the PE always has dense independent matmul work while the scalar engine
walks the exp chain of the current attention pair (keeps the HAM clock-gate
released at 2.4 GHz).
"""

import numpy as np
import ml_dtypes

import concourse.bass as bass
from concourse import bacc
import concourse.mybir as mybir
import concourse.tile as tile
from concourse import bass_utils

B, S, D = 16, 512, 768
H, DK = 12, 64
DKE = DK + 1  # head width incl. the ones column in the augmented V
VE = H * DKE  # 780
NCORES = 8
BC = B // NCORES  # batches per core
P = 128
DC = D // P  # 6 chunks of d_model
SC = S // P  # 4 chunks of sequence
NEG = np.float32(-1e9)
F32 = mybir.dt.float32
F32R = mybir.dt.float32r
BF16 = mybir.dt.bfloat16
AF = mybir.ActivationFunctionType
N_WARMUP = 16


def build_program():
    nc = bacc.Bacc()
    # fp32r: fp32-width storage the PE consumes at bf16 rate; used on the
    # score-matmul path (q/k) where exp() amplifies rounding. walrus forbids
    # mixing 32-bit and 16-bit matmul inputs, so each matmul is uniformly
    # bf16 (projections, attn@V) or uniformly f32r (scores, denom broadcast).
    MM = F32R

    xqT = nc.declare_dram_parameter("xqT", [BC, D, S], BF16, isOutput=False)
    xkT = nc.declare_dram_parameter("xkT", [BC, D, S], BF16, isOutput=False)
    xvT = nc.declare_dram_parameter("xvT", [BC, D, S], BF16, isOutput=False)
    eadjT = nc.declare_dram_parameter("eadjT", [BC, S, S], BF16, isOutput=False)
    WqT = nc.declare_dram_parameter("WqT", [D, D], BF16, isOutput=False)
    WkT = nc.declare_dram_parameter("WkT", [D, D], BF16, isOutput=False)
    WvT = nc.declare_dram_parameter("WvT", [D, VE], BF16, isOutput=False)
    WoT = nc.declare_dram_parameter("WoT", [D, D], BF16, isOutput=False)
    bqd = nc.declare_dram_parameter("bqd", [D], F32, isOutput=False)
    bkd = nc.declare_dram_parameter("bkd", [D], F32, isOutput=False)
    bvd = nc.declare_dram_parameter("bvd", [VE], F32, isOutput=False)
    bvrow = nc.declare_dram_parameter("bvrow", [1, VE], BF16, isOutput=False)
    borow = nc.declare_dram_parameter("borow", [1, D], BF16, isOutput=False)
    bod = nc.declare_dram_parameter("bod", [D], F32, isOutput=False)
    y = nc.declare_dram_parameter("y", [BC, S, D], F32, isOutput=True)

    with tile.TileContext(nc) as tc:
        with (
            tc.tile_pool(name="wpool", bufs=1) as wpool,
            tc.tile_pool(name="xpool", bufs=1) as xpool,
            tc.tile_pool(name="qkpool", bufs=4) as qkpool,
            tc.tile_pool(name="vpool", bufs=2) as vpool,
            tc.tile_pool(name="adjpool", bufs=2) as adjpool,
            tc.tile_pool(name="etpool", bufs=5) as etpool,
            tc.tile_pool(name="xopool", bufs=2) as xopool,
            tc.tile_pool(name="lpool", bufs=2) as lpool,
            tc.tile_pool(name="lbpool", bufs=2) as lbpool,
            tc.tile_pool(name="tmpool", bufs=2) as tmpool,
            tc.tile_pool(name="ypool", bufs=2) as ypool,
            # emission follows pipeline order (proj b+1 before outproj b), so
            # proj and out-proj share one psum pool without cross-batch
            # serialization; the freed bank deepens the score pipeline (sp=4)
            tc.tile_pool(name="pp", bufs=2, space="PSUM") as pp,
            tc.tile_pool(name="sp", bufs=2, space="PSUM") as sp,
            tc.tile_pool(name="xp", bufs=2, space="PSUM") as xp,
        ):
            # ---- loads, issued in need-order on the sync HW queue ----
            wv_sb = wpool.tile([P, DC, VE], BF16)
            nc.sync.dma_start(wv_sb, WvT.rearrange("(c p) e -> p c e", p=P))
            xv0_sb = xpool.tile([P, DC, S], BF16, tag="xv", name="xv_0")
            nc.sync.dma_start(xv0_sb, xvT[0].rearrange("(c p) i -> p c i", p=P))
            bvr_sb = wpool.tile([1, VE], BF16)
            nc.gpsimd.dma_start(bvr_sb, bvrow[:, :])
            bor_sb = wpool.tile([1, D], BF16)
            nc.gpsimd.dma_start(bor_sb, borow[:, :])
            onesbf = wpool.tile([1, P], F32)
            nc.vector.memset(onesbf, 1.0)
            onesb = wpool.tile([1, P], BF16)
            nc.vector.tensor_copy(onesb, onesbf)
            # whole-tensor weight loads (per-eb column chunks produced 256B
            # DMA lines, well under the efficiency knee), in need-order
            wq_sb = wpool.tile([P, DC, D], BF16)
            nc.sync.dma_start(wq_sb, WqT.rearrange("(c p) e -> p c e", p=P))
            xq0_sb = xpool.tile([P, DC, S], BF16, tag="xq", name="xq_0")
            nc.sync.dma_start(xq0_sb, xqT[0].rearrange("(c p) i -> p c i", p=P))
            wk_sb = wpool.tile([P, DC, D], BF16)
            nc.sync.dma_start(wk_sb, WkT.rearrange("(c p) e -> p c e", p=P))
            xk0_sb = xpool.tile([P, DC, S], BF16, tag="xk", name="xk_0")
            nc.sync.dma_start(xk0_sb, xkT[0].rearrange("(c p) i -> p c i", p=P))
            bq_sb = wpool.tile([P, DC], F32)
            nc.gpsimd.dma_start(bq_sb, bqd.rearrange("(c p) -> p c", p=P))
            bk_sb = wpool.tile([P, DC], F32)
            nc.gpsimd.dma_start(bk_sb, bkd.rearrange("(c p) -> p c", p=P))
            # batch-1 activations up-front on the sync queue (their slot waits
            # are off the critical path); adj + Wo ride the gpsimd SWDGE queue
            # ahead of the elementwise muls emitted later
            x1_tiles = {}
            for tag, src in (("xv", xvT), ("xq", xqT), ("xk", xkT)):
                for b1 in range(1, BC):
                    t = xpool.tile([P, DC, S], BF16, tag=tag, name=f"{tag}_{b1}")
                    nc.sync.dma_start(t, src[b1].rearrange("(c p) i -> p c i", p=P))
                    x1_tiles[(tag, b1)] = t
            adj_tiles = []
            for bb in range(BC):
                a = adjpool.tile([P, SC, S], BF16, tag="adj", name=f"adj_{bb}")
                nc.gpsimd.dma_start(a, eadjT[bb].rearrange("(c p) i -> p c i", p=P))
                adj_tiles.append(a)
            wo_sb = wpool.tile([P, DC, D], BF16)
            nc.gpsimd.dma_start(wo_sb, WoT.rearrange("(c p) e -> p c e", p=P))
            # warmup: dependency-free matmuls span the initial DMA wait so the
            # PE HAM clock-gate is released (2.4 GHz) before real matmuls
            wuf_sb = wpool.tile([P, S], F32)
            nc.vector.memset(wuf_sb, 0.0)
            wu_sb = wpool.tile([P, S], MM)
            nc.vector.tensor_copy(wu_sb, wuf_sb)
            for wi in range(N_WARMUP):
                wps = sp.tile([P, 2, S], F32, tag="s", name=f"warm_{wi}")
                nc.tensor.matmul(wps[:, 0, :], lhsT=wu_sb[:, 0:P], rhs=wu_sb, start=True, stop=True)

            # row 64 of a [65, DK] ones tile: lhsT for the K=1 broadcast of
            # the softmax denominator (matmul operand bases must match: the
            # denominator lives on partition 64 of the attn@V psum)
            ones64f_sb = wpool.tile([DKE, DK], F32)
            nc.vector.memset(ones64f_sb[DK : DK + 1, :], 1.0)
            ones64_sb = wpool.tile([DKE, DK], MM)
            nc.vector.tensor_copy(ones64_sb[DK : DK + 1, :], ones64f_sb[DK : DK + 1, :])

            # ---------------- per-batch emitters ----------------
            def setup_batch(b):
                if b == 0:
                    xv_sb, xq_sb, xk_sb = xv0_sb, xq0_sb, xk0_sb
                else:
                    xv_sb = x1_tiles[("xv", b)]
                    xq_sb = x1_tiles[("xq", b)]
                    xk_sb = x1_tiles[("xk", b)]
                return dict(
                    b=b,
                    xv=xv_sb,
                    xq=xq_sb,
                    xk=xk_sb,
                    adj=adj_tiles[b],
                    v=vpool.tile([P, SC, VE], BF16, tag="v", name=f"v_{b}"),
                    xout=xopool.tile([P, DC, S], BF16, tag="xout", name=f"xout_{b}"),
                    qts=[],
                    kts=[],
                )

            def emit_vproj_group(st, sc, hf):
                b = st["b"]
                ps_v = pp.tile([P, S], F32, tag="pp", name=f"psv_{b}_{sc}_{hf}")
                pv = ps_v[:, : VE // 2]
                for dc in range(DC):
                    nc.tensor.matmul(
                        pv,
                        lhsT=st["xv"][:, dc, sc * P : (sc + 1) * P],
                        rhs=wv_sb[:, dc, hf * (VE // 2) : (hf + 1) * (VE // 2)],
                        start=(dc == 0),
                        stop=False,
                    )
                # bias folded in as a rank-1 K=1 matmul so the copyback can
                # run on the lightly-loaded scalar engine instead of DVE
                nc.tensor.matmul(
                    pv,
                    lhsT=onesb,
                    rhs=bvr_sb[:, hf * (VE // 2) : (hf + 1) * (VE // 2)],
                    start=False,
                    stop=True,
                )
                nc.scalar.copy(
                    st["v"][:, sc, hf * (VE // 2) : (hf + 1) * (VE // 2)], pv
                )

            def emit_qk_group(st, eb):
                b = st["b"]
                ps_q = pp.tile([P, S], F32, tag="pp", name=f"psq_{b}_{eb}")
                for dc in range(DC):
                    nc.tensor.matmul(
                        ps_q,
                        lhsT=wq_sb[:, dc, eb * P : (eb + 1) * P],
                        rhs=st["xq"][:, dc, :],
                        start=(dc == 0),
                        stop=(dc == DC - 1),
                    )
                qt_c = qkpool.tile([P, S], MM, tag="qt", name=f"qt_{b}_{eb}")
                nc.scalar.activation(qt_c, ps_q, AF.Identity, bias=bq_sb[:, eb : eb + 1])
                st["qts"].append(qt_c)
                ps_k = pp.tile([P, S], F32, tag="pp", name=f"psk_{b}_{eb}")
                for dc in range(DC):
                    nc.tensor.matmul(
                        ps_k,
                        lhsT=wk_sb[:, dc, eb * P : (eb + 1) * P],
                        rhs=st["xk"][:, dc, :],
                        start=(dc == 0),
                        stop=(dc == DC - 1),
                    )
                kt_c = qkpool.tile([P, S], MM, tag="kt", name=f"kt_{b}_{eb}")
                nc.scalar.activation(kt_c, ps_k, AF.Identity, bias=bk_sb[:, eb : eb + 1])
                st["kts"].append(kt_c)

            def proj_thunks(st):
                th = []
                for sc in range(SC):
                    for hf in range(2):
                        th.append(lambda st=st, sc=sc, hf=hf: emit_vproj_group(st, sc, hf))
                for eb in range(DC):
                    th.append(lambda st=st, eb=eb: emit_qk_group(st, eb))
                return th  # 14 thunks

            def emit_scores_pair(st, ch):
                # both heads of the pair interleaved per jc chunk: the even
                # head streams on PE rows 0:64, the odd head on rows 64:128,
                # so consecutive matmuls overlap on the array
                b = st["b"]
                ets = [[None] * SC, [None] * SC]
                for jc in range(SC):
                    ps2 = sp.tile([P, 2, S], F32, tag="s", name=f"pss_{b}_{ch}_{jc}")
                    for hp in range(2):
                        po = hp * DK
                        nc.tensor.matmul(
                            ps2[:, hp, :],
                            lhsT=st["kts"][ch][po : po + DK, jc * P : (jc + 1) * P],
                            rhs=st["qts"][ch][po : po + DK, :],
                            start=True,
                            stop=True,
                        )
                    # one wide exp covers both heads' chunk (halves the number
                    # of scalar-engine instructions on the softmax pacer)
                    et2 = etpool.tile([P, 2, S], BF16, tag="et", name=f"et_{b}_{ch}_{jc}")
                    nc.scalar.activation(et2, ps2, AF.Exp)
                    for hp in range(2):
                        et = et2[:, hp, :]
                        eng = nc.gpsimd if (jc + hp) % 2 == 0 else nc.vector
                        eng.tensor_mul(et, et, st["adj"][:, jc, :])
                        ets[hp][jc] = et
                return ets

            def emit_attnv(st, h, et):
                b = st["b"]
                xps = xp.tile([DKE, S], F32, tag="x", name=f"xps_{b}_{h}")
                for jc in range(SC):
                    nc.tensor.matmul(
                        xps,
                        lhsT=st["v"][:, jc, h * DKE : (h + 1) * DKE],
                        rhs=et[jc],
                        start=(jc == 0),
                        stop=(jc == SC - 1),
                    )
                # row 64 of xps is l = sum_j E.T; broadcast it over the head's
                # 64 partitions with a K=1 matmul, then 1/l on the 64 lanes
                l_sb = lpool.tile([DKE, S], MM, tag="l", name=f"l_{b}_{h}")
                nc.vector.tensor_copy(l_sb[DK : DK + 1, :], xps[DK : DK + 1, :])
                bps = pp.tile([DK, S], F32, tag="pp", name=f"bps_{b}_{h}")
                nc.tensor.matmul(
                    bps,
                    lhsT=ones64_sb[DK : DK + 1, :],
                    rhs=l_sb[DK : DK + 1, :],
                    start=True,
                    stop=True,
                )
                linvb_sb = lbpool.tile([DK, S], F32, tag="linvb", name=f"linvb_{b}_{h}")
                nc.vector.reciprocal_approx_fast(linvb_sb, bps)
                if h % 2 == 0:
                    nc.vector.tensor_mul(
                        st["xout"][0:DK, h // 2, :], xps[0:DK, :], linvb_sb
                    )
                else:
                    tmp_sb = tmpool.tile([DK, S], BF16, tag="tmp", name=f"tmp_{b}_{h}")
                    nc.vector.tensor_mul(tmp_sb, xps[0:DK, :], linvb_sb)
                    # pack DMA on the sync queue: all input loads were
                    # triggered up-front, so its wait-for-tmp can only delay
                    # later packs/stores, which wait on the same compute anyway
                    nc.sync.dma_start(st["xout"][DK:P, h // 2, :], tmp_sb)

            def emit_outproj_group(st, ib):
                b = st["b"]
                y_sb = ypool.tile([P, D], F32, tag="y", name=f"y_{b}_{ib}")
                for hf in range(2):
                    ps_y = pp.tile([P, S], F32, tag="pp", name=f"psy_{b}_{ib}_{hf}")
                    py = ps_y[:, : D // 2]
                    for fc in range(DC):
                        nc.tensor.matmul(
                            py,
                            lhsT=st["xout"][:, fc, ib * P : (ib + 1) * P],
                            rhs=wo_sb[:, fc, hf * (D // 2) : (hf + 1) * (D // 2)],
                            start=(fc == 0),
                            stop=False,
                        )
                    nc.tensor.matmul(
                        py,
                        lhsT=onesb,
                        rhs=bor_sb[:, hf * (D // 2) : (hf + 1) * (D // 2)],
                        start=False,
                        stop=True,
                    )
                    nc.scalar.copy(y_sb[:, hf * (D // 2) : (hf + 1) * (D // 2)], py)
                nc.sync.dma_start(y[b, ib * P : (ib + 1) * P, :], y_sb)

            def emit_attention(st, filler):
                """Emit the 6 attention pairs of batch st, software-pipelined
                one pair ahead, draining `filler` thunks between pairs to give
                the PE independent work while the exp chain runs."""
                n = len(filler)
                fi = 0
                prev = None
                for ch in range(H // 2):
                    ets = emit_scores_pair(st, ch)
                    upto = (n * (ch + 1)) // (H // 2)
                    while fi < upto:
                        filler[fi]()
                        fi += 1
                    if prev is not None:
                        pch, pets = prev
                        emit_attnv(st, 2 * pch, pets[0])
                        emit_attnv(st, 2 * pch + 1, pets[1])
                    prev = (ch, ets)
                pch, pets = prev
                emit_attnv(st, 2 * pch, pets[0])
                emit_attnv(st, 2 * pch + 1, pets[1])
                while fi < n:
                    filler[fi]()
                    fi += 1

            # ---------------- phase schedule ----------------
            # phase C (b1 attention) needs more PE filler than b0's
            # out-projection alone, so 4 of b1's Q/K projection chunks are
            # deferred into it (pair ch only reads qk chunk ch, and chunk
            # 2+k is emitted after pair k, so emission stays ahead of use)
            st0 = setup_batch(0)
            for t in proj_thunks(st0):  # phase A
                t()
            st1 = setup_batch(1)
            th1 = proj_thunks(st1)
            emit_attention(st0, th1[:10])  # phase B: b0 attn + b1 V/qk01
            out0 = [lambda ib=ib: emit_outproj_group(st0, ib) for ib in range(SC)]
            emit_attention(st1, th1[10:] + out0)  # phase C: b1 attn + qk2345 + outproj0
            for ib in range(SC):  # phase D
                emit_outproj_group(st1, ib)

    nc.finalize()
    return nc


def host_prep(q, k, v, mask, adj, Wq, bq, Wk, bk, Wv, bv, Wo, bo):
    """Build per-core input maps (numpy layout prep)."""
    f = np.float32
    bf16 = ml_dtypes.bfloat16
    q = np.asarray(q, f)
    k = np.asarray(k, f)
    v = np.asarray(v, f)
    mask = np.asarray(mask, f).reshape(B, S)
    adj = np.asarray(adj, f).reshape(B, S, S)
    scale = f(1.0) / np.sqrt(f(DK))

    WqTs = np.ascontiguousarray((np.asarray(Wq, f).T * scale).astype(bf16))
    WkT = np.ascontiguousarray(np.asarray(Wk, f).T.astype(bf16))
    WoT = np.ascontiguousarray(np.asarray(Wo, f).T.astype(bf16))
    bqs = np.asarray(bq, f) * scale
    bk_ = np.asarray(bk, f)
    bo_ = np.asarray(bo, f)
    # augment Wv/bv with a zero column / 1.0 bias at e' = h*65+64 per head,
    # so the V projection emits a ones column that attn@V turns into the
    # softmax denominator
    WvT = np.zeros((D, VE), f)
    bv_ = np.zeros((VE,), f)
    WvT_nat = np.asarray(Wv, f).T
    bv_nat = np.asarray(bv, f)
    for h in range(H):
        WvT[:, h * DKE : h * DKE + DK] = WvT_nat[:, h * DK : (h + 1) * DK]
        bv_[h * DKE : h * DKE + DK] = bv_nat[h * DK : (h + 1) * DK]
        bv_[h * DKE + DK] = 1.0
    WvT = WvT.astype(bf16)

    # multiplicative scores bias, transposed: exp(adjT + NEG*mask) so the
    # device computes exp(S+adj+mask*NEG) as exp(S)*eadjT (masked keys -> 0)
    adjT = adj.transpose(0, 2, 1) + (NEG * mask)[:, :, None]
    eadjT = np.ascontiguousarray(np.exp(adjT, dtype=f).astype(bf16))

    qT = np.ascontiguousarray(q.transpose(0, 2, 1).astype(bf16))
    kT = np.ascontiguousarray(k.transpose(0, 2, 1).astype(bf16))
    vT = np.ascontiguousarray(v.transpose(0, 2, 1).astype(bf16))

    in_maps = []
    for c in range(NCORES):
        sl = slice(c * BC, (c + 1) * BC)
        in_maps.append(
            {
                "xqT": qT[sl],
                "xkT": kT[sl],
                "xvT": vT[sl],
                "eadjT": eadjT[sl],
                "WqT": WqTs,
                "WkT": WkT,
                "WvT": WvT,
                "WoT": WoT,
                "bqd": bqs,
                "bkd": bk_,
                "bvd": bv_,
                "bvrow": bv_.astype(bf16)[None, :],
                "borow": bo_.astype(bf16)[None, :],
                "bod": bo_,
            }
        )
    return in_maps


_PROGRAM = None


def _get_program():
    global _PROGRAM
    if _PROGRAM is None:
        _PROGRAM = build_program()
    return _PROGRAM


def kernel(q, k, v, mask, adj, Wq, bq, Wk, bk, Wv, bv, Wo, bo):
    nc = _get_program()
    in_maps = host_prep(q, k, v, mask, adj, Wq, bq, Wk, bk, Wv, bv, Wo, bo)
    res = bass_utils.run_bass_kernel_spmd(nc, in_maps, list(range(NCORES)))
    out = np.concatenate([np.asarray(res.results[i]["y"]) for i in range(NCORES)], axis=0)
    return out.astype(np.float32)
